# revision 24
# baseline (speedup 1.0000x reference)
"""Trainium2 Bass kernel for single-head attention:
out = softmax((x@Wq)(x@Wk)^T / sqrt(D)) (x@Wv) @ Wout + bout, per batch.

Shapes: x [4, 2048, 1024], Wqkv [1024, 3072], Wout [1024, 1024].

Weight folding (host-side, exact):
  logits = (x Wq + bq)(x Wk + bk)^T * s
         = x (s Wq Wk^T) x^T + (s bq Wk^T) x^T + per-query-consts
  The per-query constant terms cancel in softmax, so with
  M = s*Wq@Wk^T and zbias = s*bq@Wk^T:  logits ~ (x@M + zbias) @ x^T.
  Similarly out = A (x Wv + bv) Wout + bout with A row-stochastic gives
  out = (P @ x @ U) / rowsum(P) + bprime, U = Wv@Wout, bprime = bv@Wout+bout.
This removes the K and V projections and the separate out-projection:
12.9 GFLOP/core (768 fp16-equivalent matmuls) instead of 21.5 GFLOP/core.

Sharding: 8 cores = 4 batches x 2 query-halves. Each core gets the full
sequence of its batch (rotated so its 1024 queries are rows 0:1023 --
attention is permutation-invariant over key/value positions) and writes
a [1024, 1024] slice of the output. No collectives.

All inputs are host-packed into the exact SBUF layouts so every load is
one dma_start with fully-contiguous per-partition reads. 17 HAM-warmup
matmuls bridge the ~13us until the first inputs land (the DMA engines
ramp slowly: ~50 GB/s for the first ~10us, ~350 GB/s after).

Device phases (matmuls fp16 with fp32 PSUM accumulation, except the
fp8 part of C):
  A: zT[d',s]  = M^T x_local^T        (128 mm)
  B: PT[t,s]   = exp(x z^T + shift)   (256 mm), running DVE row-sum acc;
     for the first N_C8_ST key-tiles a second exp (bias shift+ln16)
     writes PT8 = e4m3(P*16) straight from PSUM
  C: YT[d,s]   = x-contracted P: the first N_C8_ST key-tiles as fp8
     DoubleRow matmuls (contract 2 key-tiles per instruction at fp16
     cost -- 2x) against xn8 = e4m3(x/16) into a separate PSUM bank
     (mixing perf modes inside one accumulation group crashes the PE;
     a partial-inner-width DoubleRow rhs slice deadlocks scheduling, so
     PT8 is stored per-sc full-width), rest fp16; ACT stages the fp8
     psum to SBUF, DVE adds it into the YT write. PT8*xn8 = P*x
     natively, so fp8/fp16 partials add without rescaling. The DR groups
     use a dedicated 2-bank PSUM pool so their group-start allocation
     is always freed by the fast ACT copy, not a slow DVE add.
  D: out[s,o]  = (YT^T U) * rsum + bprime             (128 mm)
Softmax exp uses a constant -4 logit shift so fp16 never overflows
(the shift cancels in the normalization); rowsum uses the fp16 PT
(e4m3 rounding is ~unbiased, the mismatch is ~0.1% of the fp8 noise).

N_C8_ST=6 gives rel err 1.66e-2 (vs 6e-4 all-fp16, 2e-2 budget),
measured bit-matching the numpy e4m3 simulation, and ~10us of PE time.
Outputs alternate the sync/scalar DMA queues; the last tile's STT and
DMA go in quarters so the exposed final chain is short.

Observed hazard: the chip sporadically enters power state P0 (whole
core -1/6 clock, 216->259ns per matmul) for a full run -- environmental
and not caused by this program (same binary measured 177us and 211us).
"""

import sys

if "/opt/trn_rl_repo" not in sys.path:
    sys.path.insert(0, "/opt/trn_rl_repo")

import numpy as np

import concourse.mybir as mybir
from concourse import bacc
from concourse.tile import TileContext

P = 128
D = 1024          # d_model
S = 2048          # full sequence per batch
SQ = 1024         # queries per core
DC = D // P       # 8 d-chunks
ST = S // P       # 16 sequence tiles
NQ = SQ // 512    # 2 query 512-chunks
SQT = SQ // P     # 8 query 128-tiles

F32 = mybir.dt.float32
F16 = mybir.dt.float16
F8 = mybir.dt.float8e4
EXP_SHIFT = -4.0  # softmax logit shift (cancels in normalization)

# key st-tiles of phase C computed in fp8 DoubleRow (must be even).
# rel-err (device-exact sim): 0->6e-4, 2->9.7e-3, 4->1.35e-2, 6->1.65e-2
N_C8_ST = 6
P8_SCALE = 16.0   # PT8 = e4m3(P*16), xn8 = e4m3(x/16)


def _build_core_program():
    nc = bacc.Bacc()

    # host-packed inputs: leading axis 128 = SBUF partition, rest contiguous
    xta_d = nc.dram_tensor("xta", [P, DC, 512], F16, kind="ExternalInput")
    xtb_d = nc.dram_tensor("xtb", [P, DC, 512], F16, kind="ExternalInput")
    xtc_d = nc.dram_tensor("xtc", [P, DC, 1024], F16, kind="ExternalInput")
    xn_d = nc.dram_tensor("xn16", [S, D], F16, kind="ExternalInput")
    mj0_d = nc.dram_tensor("mj0", [P, DC, P], F16, kind="ExternalInput")
    mj1_d = nc.dram_tensor("mj1", [P, DC, P], F16, kind="ExternalInput")
    mj23_d = nc.dram_tensor("mj23", [P, DC, 2 * P], F16, kind="ExternalInput")
    mj47_d = nc.dram_tensor("mj47", [P, DC, 4 * P], F16, kind="ExternalInput")
    u_d = nc.dram_tensor("U16p", [2, P, DC, 512], F16, kind="ExternalInput")
    zb_d = nc.dram_tensor("zbias", [D], F32, kind="ExternalInput")
    bp_d = nc.dram_tensor("bprime", [D], F32, kind="ExternalInput")
    if N_C8_ST:
        xn8_d = nc.dram_tensor(
            "xn8", [P, DC, N_C8_ST, P], F8, kind="ExternalInput"
        )
    out_d = nc.dram_tensor("out", [SQ, D], F16, kind="ExternalOutput")

    with TileContext(nc) as tc:
        with (
            tc.tile_pool(name="const", bufs=1) as const,
            tc.tile_pool(name="ps_mm", bufs=5, space="PSUM") as ps_mm,
            tc.tile_pool(name="ps_c8", bufs=2, space="PSUM") as ps_c8,
            tc.tile_pool(name="ps_sum", bufs=1, space="PSUM") as ps_sum,
            tc.tile_pool(name="dramtmp", bufs=1, space="DRAM") as dramtmp,
        ):
            # warm memset on GpSimd (ready ~1us before DVE) so HAM warmup
            # matmuls start ASAP
            warm = const.tile([P, 512], F16)
            nc.gpsimd.memset(warm, 0.0)
            ones16 = const.tile([P, 1], F16)
            nc.vector.memset(ones16, 1.0)
            shift_b = const.tile([P, 1], F32)
            nc.vector.memset(shift_b, EXP_SHIFT)
            shift8_b = const.tile([P, 1], F32)
            nc.vector.memset(shift8_b, EXP_SHIFT + float(np.log(P8_SCALE)))
            sums_sb = const.tile([1, SQ], F32)
            sumsT = const.tile([P, SQT], F32)
            rsum = const.tile([P, SQT], F32)

            with tc.tile_pool(name="big", bufs=1) as big:
                xta = big.tile([P, DC, 512], F16)
                xtb = big.tile([P, DC, 512], F16)
                xtc = big.tile([P, DC, 1024], F16)
                xn = big.tile([P, ST, D], F16)
                m_t = big.tile([P, DC, D], F16)
                u16 = big.tile([P, 2, DC, 512], F16)
                zb = const.tile([P, DC], F32)
                bp_b = const.tile([P, D], F32)
                zT = big.tile([P, DC, SQ], F16)
                PT = [big.tile([P, SQ], F16, name=f"PT{st}") for st in range(ST)]
                acc = big.tile([P, SQ], F16, name="pt_acc")
                YT = big.tile([P, DC, SQ], F16)
                if N_C8_ST:
                    xn8 = big.tile([P, DC, N_C8_ST, P], F8)
                    yt8tmp = [
                        big.tile([P, 512], F32, name=f"yt8tmp{i}")
                        for i in range(2 * NQ)
                    ]
                    PT8s = [
                        big.tile([P, N_C8_ST, 512], F8, name=f"PT8_{sc}")
                        for sc in range(NQ)
                    ]

                # ---- input DMAs, ordered for earliest compute start --------
                # first M chunk + late-needed consts on the gpsimd queue,
                # everything else on sync in consumption order
                nc.sync.dma_start(out=m_t[:, :, 0:P], in_=mj0_d[:, :, :])
                nc.sync.dma_start(out=xta, in_=xta_d[:, :, :])
                nc.sync.dma_start(
                    out=zb, in_=zb_d.rearrange("(j p) -> p j", p=P)
                )
                nc.sync.dma_start(out=m_t[:, :, P : 2 * P], in_=mj1_d[:, :, :])
                nc.sync.dma_start(out=m_t[:, :, 2 * P : 4 * P], in_=mj23_d[:, :, :])
                nc.sync.dma_start(out=m_t[:, :, 4 * P : 8 * P], in_=mj47_d[:, :, :])
                nc.sync.dma_start(out=xtb, in_=xtb_d[:, :, :])
                nc.sync.dma_start(out=xtc, in_=xtc_d[:, :, :])
                for g in range(4):
                    nc.sync.dma_start(
                        out=xn[:, g * 4 : (g + 1) * 4, :],
                        in_=xn_d[g * 512 : (g + 1) * 512, :].rearrange(
                            "(st p) d -> p st d", p=P
                        ),
                    )
                if N_C8_ST:
                    nc.sync.dma_start(out=xn8, in_=xn8_d[:, :, :, :])
                for oc in range(2):
                    nc.sync.dma_start(out=u16[:, oc, :, :], in_=u_d[oc, :, :, :])
                # 512KB broadcast write hogs one DMA engine for ~16us --
                # keep it out of the critical early window (needed in D only)
                nc.sync.dma_start(
                    out=bp_b, in_=bp_d[None, :].to_broadcast([P, D])
                )

                # HAM warmup: keep the PE busy while the first inputs stream
                # in, so real matmuls start at 2.4 GHz instead of 1.2 GHz
                ps_warms = [
                    ps_mm.tile([P, 512], F32, tag="mm", name="ps_warm")
                    for _ in range(2)
                ]
                for i in range(17):
                    nc.tensor.matmul(
                        ps_warms[i % 2],
                        lhsT=warm[:, 0:P],
                        rhs=warm,
                        start=True,
                        stop=True,
                    )

                def xt_lhs(dc, st):
                    """lhsT slice [128 d, 128 t] of x^T for key tile st."""
                    if st < 4:
                        return xta[:, dc, st * P : (st + 1) * P]
                    if st < 8:
                        return xtb[:, dc, (st - 4) * P : (st - 3) * P]
                    return xtc[:, dc, (st - 8) * P : (st - 7) * P]

                # ======== phase A: zT[d', s] = M^T xq^T + zbias ============
                # sc outer: the first 64 matmuls touch only xta
                for sc in range(NQ):
                    xsrc = xta if sc == 0 else xtb
                    for j in range(DC):
                        ps = ps_mm.tile([P, 512], F32, tag="mm", name="mm")
                        for dc in range(DC):
                            nc.tensor.matmul(
                                ps,
                                lhsT=m_t[:, dc, j * P : (j + 1) * P],
                                rhs=xsrc[:, dc, :],
                                start=(dc == 0),
                                stop=(dc == DC - 1),
                            )
                        nc.scalar.activation(
                            zT[:, j, sc * 512 : (sc + 1) * 512],
                            ps,
                            mybir.ActivationFunctionType.Identity,
                            bias=zb[:, j : j + 1],
                            scale=1.0,
                        )

                # ======== phase B: PT[t, s] = exp(x z^T + shift) ===========
                for st in range(ST):
                    pss = [
                        ps_mm.tile([P, 512], F32, tag="mm", name="mm") for _ in range(NQ)
                    ]
                    for dc in range(DC):
                        for sc in range(NQ):
                            nc.tensor.matmul(
                                pss[sc],
                                lhsT=xt_lhs(dc, st),
                                rhs=zT[:, dc, sc * 512 : (sc + 1) * 512],
                                start=(dc == 0),
                                stop=(dc == DC - 1),
                            )
                    for sc in range(NQ):
                        nc.scalar.activation(
                            PT[st][:, sc * 512 : (sc + 1) * 512],
                            pss[sc],
                            mybir.ActivationFunctionType.Exp,
                            bias=shift_b[:, 0:1],
                            scale=1.0,
                        )
                    # fp8 PT (P*16 via exp bias shift+ln16, straight from
                    # PSUM) for the key-tiles phase C contracts in
                    # DoubleRow mode; the rowsum acc keeps using the fp16
                    # PT (quantization is unbiased; mismatch ~0.1%)
                    if st < N_C8_ST:
                        for sc in range(NQ):
                            nc.scalar.activation(
                                PT8s[sc][:, st, :],
                                pss[sc],
                                mybir.ActivationFunctionType.Exp,
                                bias=shift8_b[:, 0:1],
                                scale=1.0,
                            )
                    # running elementwise sum of PT tiles on the (idle) DVE
                    if st == 1:
                        nc.vector.tensor_tensor(
                            out=acc, in0=PT[0], in1=PT[1], op=mybir.AluOpType.add
                        )
                    elif st > 1:
                        nc.vector.tensor_tensor(
                            out=acc, in0=acc, in1=PT[st], op=mybir.AluOpType.add
                        )
                # ======== phase C: YT[d, s] = x-contracted P ===============
                # (row-sum pipeline emitted after C's first group so the two
                # ones-matmuls don't sit on the B->C critical path)
                for dc in range(DC):
                    pss = [
                        ps_mm.tile([P, 512], F32, tag="mm", name="mm") for _ in range(NQ)
                    ]
                    if N_C8_ST:
                        # dedicated banks: freed by the (fast) ACT copies so
                        # the next group's DR start never waits on the slower
                        # DVE-add-freed fp16 banks
                        ps8 = [
                            ps_c8.tile([P, 512], F32, tag="mm8", name="mm8")
                            for _ in range(NQ)
                        ]
                        for sp in range(N_C8_ST // 2):
                            for sc in range(NQ):
                                nc.tensor.matmul(
                                    ps8[sc],
                                    lhsT=xn8[:, dc, 2 * sp : 2 * sp + 2, :],
                                    rhs=PT8s[sc][:, 2 * sp : 2 * sp + 2, :],
                                    start=(sp == 0),
                                    stop=(sp == N_C8_ST // 2 - 1),
                                    perf_mode=mybir.MatmulPerfMode.DoubleRow,
                                )
                        for sc in range(NQ):
                            # idle ACT engine does the psum->sbuf stage so
                            # DVE only has the adds after each group stop
                            nc.scalar.activation(
                                yt8tmp[NQ * (dc % 2) + sc],
                                ps8[sc],
                                mybir.ActivationFunctionType.Identity,
                                scale=1.0,
                            )
                    for st in range(N_C8_ST, ST):
                        for sc in range(NQ):
                            nc.tensor.matmul(
                                pss[sc],
                                lhsT=xn[:, st, dc * P : (dc + 1) * P],
                                rhs=PT[st][:, sc * 512 : (sc + 1) * 512],
                                start=(st == N_C8_ST),
                                stop=(st == ST - 1),
                            )
                    for sc in range(NQ):
                        if N_C8_ST:
                            nc.vector.tensor_tensor(
                                out=YT[:, dc, sc * 512 : (sc + 1) * 512],
                                in0=pss[sc],
                                in1=yt8tmp[NQ * (dc % 2) + sc],
                                op=mybir.AluOpType.add,
                            )
                        else:
                            nc.vector.tensor_copy(
                                out=YT[:, dc, sc * 512 : (sc + 1) * 512],
                                in_=pss[sc],
                            )
                    if dc <= 1:
                        # cross-partition query totals: one tiny ones-matmul
                        # per 512-chunk (single psum-sum bank, reused one dc
                        # group apart so the reuse never stalls the PE), then
                        # [1, SQ] -> [128, SQT] via DRAM so sums line up with
                        # out partitions (SBUF APs cannot cross partitions)
                        sc = dc
                        pssum = ps_sum.tile([1, 512], F32, name="psum_s")
                        nc.tensor.matmul(
                            pssum,
                            lhsT=ones16,
                            rhs=acc[:, sc * 512 : (sc + 1) * 512],
                            start=True,
                            stop=True,
                        )
                        nc.vector.tensor_copy(
                            out=sums_sb[:, sc * 512 : (sc + 1) * 512],
                            in_=pssum,
                        )
                    if dc == 1:
                        sums_dram = dramtmp.tile([SQ], F32)
                        nc.sync.dma_start(out=sums_dram[None, :], in_=sums_sb)
                        nc.sync.dma_start(
                            out=sumsT, in_=sums_dram.rearrange("(t p) -> p t", p=P)
                        )
                        nc.vector.reciprocal(rsum, sumsT)

                # ======== phase D: out[s, o] = (YT^T U) * rsum + bprime ====
                o_sbs = [big.tile([P, D], F16, name=f"o_sb{i}") for i in range(2)]
                for sqt in range(SQT):
                    o_sb = o_sbs[sqt % 2]
                    pss = [
                        ps_mm.tile([P, 512], F32, tag="mm", name="mm")
                        for _ in range(2)
                    ]
                    for dc in range(DC):
                        for oc in range(2):
                            nc.tensor.matmul(
                                pss[oc],
                                lhsT=YT[:, dc, sqt * P : (sqt + 1) * P],
                                rhs=u16[:, oc, dc, :],
                                start=(dc == 0),
                                stop=(dc == DC - 1),
                            )
                    n_stt = 4 if sqt == SQT - 1 else 2
                    w_stt = D // n_stt
                    for ch in range(n_stt):
                        lo = ch * w_stt
                        nc.vector.scalar_tensor_tensor(
                            out=o_sb[:, lo : lo + w_stt],
                            in0=pss[lo // 512][:, lo % 512 : lo % 512 + w_stt],
                            scalar=rsum[:, sqt : sqt + 1],
                            in1=bp_b[:, lo : lo + w_stt],
                            op0=mybir.AluOpType.mult,
                            op1=mybir.AluOpType.add,
                        )
                    # alternate output queues so tiles drain on two engines
                    # in parallel; the last group goes in quarters so its
                    # exposed final transfer is half as long
                    for oc in range(2):
                        if sqt == SQT - 1:
                            for h in range(2):
                                q = nc.sync if (2 * oc + h) % 2 == 0 else nc.scalar
                                lo = oc * 512 + h * 256
                                q.dma_start(
                                    out=out_d[
                                        sqt * P : (sqt + 1) * P, lo : lo + 256
                                    ],
                                    in_=o_sb[:, lo : lo + 256],
                                )
                        else:
                            q = nc.sync if oc == 0 else nc.scalar
                            q.dma_start(
                                out=out_d[
                                    sqt * P : (sqt + 1) * P, oc * 512 : (oc + 1) * 512
                                ],
                                in_=o_sb[:, oc * 512 : (oc + 1) * 512],
                            )

    nc.finalize()
    return nc


def kernel(x, Wqkv, bqkv, Wout, bout):
    from concourse.bass_utils import run_bass_kernel_spmd
    import ml_dtypes

    x = np.ascontiguousarray(x, dtype=np.float32)
    Wqkv = np.asarray(Wqkv, dtype=np.float32)
    bqkv = np.asarray(bqkv, dtype=np.float32)
    Wout = np.asarray(Wout, dtype=np.float32)
    bout = np.asarray(bout, dtype=np.float32)
    B = x.shape[0]
    scale = float(D) ** -0.5

    Wq, Wk, Wv = Wqkv[:, 0:D], Wqkv[:, D : 2 * D], Wqkv[:, 2 * D : 3 * D]
    bq, bk, bv = bqkv[0:D], bqkv[D : 2 * D], bqkv[2 * D : 3 * D]

    # folded weights (the dropped per-query logit terms cancel in softmax)
    M16 = (scale * (Wq @ Wk.T)).astype(np.float16)
    U16 = (Wv @ Wout).astype(np.float16)
    zbias = np.ascontiguousarray(scale * (bq @ Wk.T), dtype=np.float32)
    bprime = np.ascontiguousarray(bv @ Wout + bout, dtype=np.float32)

    # pack into per-partition-contiguous SBUF layouts
    Mp = M16.reshape(DC, P, DC, P).transpose(1, 0, 2, 3)  # (p, dc, jb, jw)
    mj0 = np.ascontiguousarray(Mp[:, :, 0, :])
    mj1 = np.ascontiguousarray(Mp[:, :, 1, :])
    mj23 = np.ascontiguousarray(Mp[:, :, 2:4, :].reshape(P, DC, 2 * P))
    mj47 = np.ascontiguousarray(Mp[:, :, 4:8, :].reshape(P, DC, 4 * P))
    U16p = np.ascontiguousarray(
        U16.reshape(DC, P, 2, 512).transpose(2, 1, 0, 3)
    )  # (oc, p, dc, ow)

    nc = _build_core_program()

    in_maps = []
    for c in range(8):
        b, h = c // 2, c % 2
        # rotate the sequence so this core's queries are rows 0:SQ
        xs = np.concatenate([x[b, h * SQ :], x[b, : h * SQ]], axis=0)
        xn16 = np.ascontiguousarray(xs.astype(np.float16))
        xT = xn16.T  # [D, S]
        im = {
                "xta": np.ascontiguousarray(
                    xT[:, 0:512].reshape(DC, P, 512).transpose(1, 0, 2)
                ),
                "xtb": np.ascontiguousarray(
                    xT[:, 512:1024].reshape(DC, P, 512).transpose(1, 0, 2)
                ),
                "xtc": np.ascontiguousarray(
                    xT[:, 1024:2048].reshape(DC, P, 1024).transpose(1, 0, 2)
                ),
                "xn16": xn16,
                "mj0": mj0,
                "mj1": mj1,
                "mj23": mj23,
                "mj47": mj47,
                "U16p": U16p,
                "zbias": zbias,
                "bprime": bprime,
        }
        if N_C8_ST:
            x8 = np.clip(
                xs[: N_C8_ST * P].astype(np.float32) / P8_SCALE, -240.0, 240.0
            ).astype(ml_dtypes.float8_e4m3)
            im["xn8"] = np.ascontiguousarray(
                x8.reshape(N_C8_ST, P, DC, P).transpose(1, 2, 0, 3)
            )
        in_maps.append(im)

    res = run_bass_kernel_spmd(nc, in_maps, core_ids=list(range(8)))

    out = np.empty((B, S, D), dtype=np.float32)
    for c in range(8):
        b, h = c // 2, c % 2
        out[b, h * SQ : (h + 1) * SQ, :] = res.results[c]["out"].astype(np.float32)
    return out



# revision 25
# speedup vs baseline: 1.0018x; 1.0018x over previous
"""Trainium2 Bass kernel for single-head attention:
out = softmax((x@Wq)(x@Wk)^T / sqrt(D)) (x@Wv) @ Wout + bout, per batch.

Shapes: x [4, 2048, 1024], Wqkv [1024, 3072], Wout [1024, 1024].

Weight folding (host-side, exact):
  logits = (x Wq + bq)(x Wk + bk)^T * s
         = x (s Wq Wk^T) x^T + (s bq Wk^T) x^T + per-query-consts
  The per-query constant terms cancel in softmax, so with
  M = s*Wq@Wk^T and zbias = s*bq@Wk^T:  logits ~ (x@M + zbias) @ x^T.
  Similarly out = A (x Wv + bv) Wout + bout with A row-stochastic gives
  out = (P @ x @ U) / rowsum(P) + bprime, U = Wv@Wout, bprime = bv@Wout+bout.
This removes the K and V projections and the separate out-projection:
12.9 GFLOP/core (768 fp16-equivalent matmuls) instead of 21.5 GFLOP/core.

Sharding: 8 cores = 4 batches x 2 query-halves. Each core gets the full
sequence of its batch (rotated so its 1024 queries are rows 0:1023 --
attention is permutation-invariant over key/value positions) and writes
a [1024, 1024] slice of the output. No collectives.

All inputs are host-packed into the exact SBUF layouts so every load is
one dma_start with fully-contiguous per-partition reads. 17 HAM-warmup
matmuls bridge the ~13us until the first inputs land (the DMA engines
ramp slowly: ~50 GB/s for the first ~10us, ~350 GB/s after).

Device phases (matmuls fp16 with fp32 PSUM accumulation, except the
fp8 part of C):
  A: zT[d',s]  = M^T x_local^T        (128 mm)
  B: PT[t,s]   = exp(x z^T + shift)   (256 mm), running DVE row-sum acc;
     for the first N_C8_ST key-tiles a second exp (bias shift+ln16)
     writes PT8 = e4m3(P*16) straight from PSUM
  C: YT[d,s]   = x-contracted P: the first N_C8_ST key-tiles as fp8
     DoubleRow matmuls (contract 2 key-tiles per instruction at fp16
     cost -- 2x) against xn8 = e4m3(x/16) into a separate PSUM bank
     (mixing perf modes inside one accumulation group crashes the PE;
     a partial-inner-width DoubleRow rhs slice deadlocks scheduling, so
     PT8 is stored per-sc full-width), rest fp16; ACT stages the fp8
     psum to SBUF, DVE adds it into the YT write. PT8*xn8 = P*x
     natively, so fp8/fp16 partials add without rescaling. The DR groups
     use a dedicated 2-bank PSUM pool so their group-start allocation
     is always freed by the fast ACT copy, not a slow DVE add.
  D: out[s,o]  = (YT^T U) * rsum + bprime             (128 mm)
Softmax exp uses a constant -4 logit shift so fp16 never overflows
(the shift cancels in the normalization); rowsum uses the fp16 PT
(e4m3 rounding is ~unbiased, the mismatch is ~0.1% of the fp8 noise).

N_C8_ST=6 gives rel err 1.66e-2 (vs 6e-4 all-fp16, 2e-2 budget),
measured bit-matching the numpy e4m3 simulation, and ~10us of PE time.
Outputs alternate the sync/scalar DMA queues; the last tile's STT and
DMA go in quarters so the exposed final chain is short.

Observed hazard: the chip sporadically enters power state P0 (whole
core -1/6 clock, 216->259ns per matmul) for a full run -- environmental
and not caused by this program (same binary measured 177us and 211us).
"""

import sys

if "/opt/trn_rl_repo" not in sys.path:
    sys.path.insert(0, "/opt/trn_rl_repo")

import numpy as np

import concourse.mybir as mybir
from concourse import bacc
from concourse.tile import TileContext

P = 128
D = 1024          # d_model
S = 2048          # full sequence per batch
SQ = 1024         # queries per core
DC = D // P       # 8 d-chunks
ST = S // P       # 16 sequence tiles
NQ = SQ // 512    # 2 query 512-chunks
SQT = SQ // P     # 8 query 128-tiles

F32 = mybir.dt.float32
F16 = mybir.dt.float16
F8 = mybir.dt.float8e4
EXP_SHIFT = -4.0  # softmax logit shift (cancels in normalization)

# key st-tiles of phase C computed in fp8 DoubleRow (must be even).
# rel-err (device-exact sim): 0->6e-4, 2->9.7e-3, 4->1.35e-2, 6->1.65e-2
N_C8_ST = 6
P8_SCALE = 16.0   # PT8 = e4m3(P*16), xn8 = e4m3(x/16)


def _build_core_program():
    nc = bacc.Bacc()

    # host-packed inputs: leading axis 128 = SBUF partition, rest contiguous
    xta_d = nc.dram_tensor("xta", [P, DC, 512], F16, kind="ExternalInput")
    xtb_d = nc.dram_tensor("xtb", [P, DC, 512], F16, kind="ExternalInput")
    xtc_d = nc.dram_tensor("xtc", [P, DC, 1024], F16, kind="ExternalInput")
    xn_d = nc.dram_tensor("xn16", [S, D], F16, kind="ExternalInput")
    mj0_d = nc.dram_tensor("mj0", [P, DC, P], F16, kind="ExternalInput")
    mj1_d = nc.dram_tensor("mj1", [P, DC, P], F16, kind="ExternalInput")
    mj23_d = nc.dram_tensor("mj23", [P, DC, 2 * P], F16, kind="ExternalInput")
    mj47_d = nc.dram_tensor("mj47", [P, DC, 4 * P], F16, kind="ExternalInput")
    u_d = nc.dram_tensor("U16p", [2, P, DC, 512], F16, kind="ExternalInput")
    zb_d = nc.dram_tensor("zbias", [D], F32, kind="ExternalInput")
    bp_d = nc.dram_tensor("bprime", [D], F32, kind="ExternalInput")
    if N_C8_ST:
        xn8_d = nc.dram_tensor(
            "xn8", [P, DC, N_C8_ST, P], F8, kind="ExternalInput"
        )
    out_d = nc.dram_tensor("out", [SQ, D], F16, kind="ExternalOutput")

    with TileContext(nc) as tc:
        with (
            tc.tile_pool(name="const", bufs=1) as const,
            tc.tile_pool(name="ps_mm", bufs=5, space="PSUM") as ps_mm,
            tc.tile_pool(name="ps_c8", bufs=2, space="PSUM") as ps_c8,
            tc.tile_pool(name="ps_sum", bufs=1, space="PSUM") as ps_sum,
            tc.tile_pool(name="dramtmp", bufs=1, space="DRAM") as dramtmp,
        ):
            # warm memset on GpSimd (ready ~1us before DVE) so HAM warmup
            # matmuls start ASAP
            warm = const.tile([P, 512], F16)
            nc.gpsimd.memset(warm, 0.0)
            ones16 = const.tile([P, 1], F16)
            nc.vector.memset(ones16, 1.0)
            shift_b = const.tile([P, 1], F32)
            nc.vector.memset(shift_b, EXP_SHIFT)
            shift8_b = const.tile([P, 1], F32)
            nc.vector.memset(shift8_b, EXP_SHIFT + float(np.log(P8_SCALE)))
            sums_sb = const.tile([1, SQ], F32)
            sumsT = const.tile([P, SQT], F32)
            rsum = const.tile([P, SQT], F32)

            with tc.tile_pool(name="big", bufs=1) as big:
                xta = big.tile([P, DC, 512], F16)
                xtb = big.tile([P, DC, 512], F16)
                xtc = big.tile([P, DC, 1024], F16)
                xn = big.tile([P, ST, D], F16)
                m_t = big.tile([P, DC, D], F16)
                u16 = big.tile([P, 2, DC, 512], F16)
                zb = const.tile([P, DC], F32)
                bp_b = const.tile([P, D], F32)
                zT = big.tile([P, DC, SQ], F16)
                PT = [big.tile([P, SQ], F16, name=f"PT{st}") for st in range(ST)]
                acc = big.tile([P, SQ], F16, name="pt_acc")
                YT = big.tile([P, DC, SQ], F16)
                if N_C8_ST:
                    xn8 = big.tile([P, DC, N_C8_ST, P], F8)
                    yt8sb = [
                        big.tile([P, 512], F16, name=f"yt8sb{i}")
                        for i in range(DC * NQ)
                    ]
                    PT8s = [
                        big.tile([P, N_C8_ST, 512], F8, name=f"PT8_{sc}")
                        for sc in range(NQ)
                    ]

                # ---- input DMAs, ordered for earliest compute start --------
                # first M chunk + late-needed consts on the gpsimd queue,
                # everything else on sync in consumption order
                nc.sync.dma_start(out=m_t[:, :, 0:P], in_=mj0_d[:, :, :])
                nc.sync.dma_start(out=xta, in_=xta_d[:, :, :])
                nc.sync.dma_start(
                    out=zb, in_=zb_d.rearrange("(j p) -> p j", p=P)
                )
                nc.sync.dma_start(out=m_t[:, :, P : 2 * P], in_=mj1_d[:, :, :])
                nc.sync.dma_start(out=m_t[:, :, 2 * P : 4 * P], in_=mj23_d[:, :, :])
                nc.sync.dma_start(out=m_t[:, :, 4 * P : 8 * P], in_=mj47_d[:, :, :])
                nc.sync.dma_start(out=xtb, in_=xtb_d[:, :, :])
                nc.sync.dma_start(out=xtc, in_=xtc_d[:, :, :])
                for g in range(4):
                    nc.sync.dma_start(
                        out=xn[:, g * 4 : (g + 1) * 4, :],
                        in_=xn_d[g * 512 : (g + 1) * 512, :].rearrange(
                            "(st p) d -> p st d", p=P
                        ),
                    )
                if N_C8_ST:
                    nc.sync.dma_start(out=xn8, in_=xn8_d[:, :, :, :])
                for oc in range(2):
                    nc.sync.dma_start(out=u16[:, oc, :, :], in_=u_d[oc, :, :, :])
                # 512KB broadcast write hogs one DMA engine for ~16us --
                # keep it out of the critical early window (needed in D only)
                nc.sync.dma_start(
                    out=bp_b, in_=bp_d[None, :].to_broadcast([P, D])
                )

                # HAM warmup: keep the PE busy while the first inputs stream
                # in, so real matmuls start at 2.4 GHz instead of 1.2 GHz
                ps_warms = [
                    ps_mm.tile([P, 512], F32, tag="mm", name="ps_warm")
                    for _ in range(2)
                ]
                for i in range(17):
                    nc.tensor.matmul(
                        ps_warms[i % 2],
                        lhsT=warm[:, 0:P],
                        rhs=warm,
                        start=True,
                        stop=True,
                    )

                def xt_lhs(dc, st):
                    """lhsT slice [128 d, 128 t] of x^T for key tile st."""
                    if st < 4:
                        return xta[:, dc, st * P : (st + 1) * P]
                    if st < 8:
                        return xtb[:, dc, (st - 4) * P : (st - 3) * P]
                    return xtc[:, dc, (st - 8) * P : (st - 7) * P]

                # ======== phase A: zT[d', s] = M^T xq^T + zbias ============
                # sc outer: the first 64 matmuls touch only xta
                for sc in range(NQ):
                    xsrc = xta if sc == 0 else xtb
                    for j in range(DC):
                        ps = ps_mm.tile([P, 512], F32, tag="mm", name="mm")
                        for dc in range(DC):
                            nc.tensor.matmul(
                                ps,
                                lhsT=m_t[:, dc, j * P : (j + 1) * P],
                                rhs=xsrc[:, dc, :],
                                start=(dc == 0),
                                stop=(dc == DC - 1),
                            )
                        nc.scalar.activation(
                            zT[:, j, sc * 512 : (sc + 1) * 512],
                            ps,
                            mybir.ActivationFunctionType.Identity,
                            bias=zb[:, j : j + 1],
                            scale=1.0,
                        )

                # ======== phase B: PT[t, s] = exp(x z^T + shift) ===========
                for st in range(ST):
                    pss = [
                        ps_mm.tile([P, 512], F32, tag="mm", name="mm") for _ in range(NQ)
                    ]
                    for dc in range(DC):
                        for sc in range(NQ):
                            nc.tensor.matmul(
                                pss[sc],
                                lhsT=xt_lhs(dc, st),
                                rhs=zT[:, dc, sc * 512 : (sc + 1) * 512],
                                start=(dc == 0),
                                stop=(dc == DC - 1),
                            )
                    for sc in range(NQ):
                        nc.scalar.activation(
                            PT[st][:, sc * 512 : (sc + 1) * 512],
                            pss[sc],
                            mybir.ActivationFunctionType.Exp,
                            bias=shift_b[:, 0:1],
                            scale=1.0,
                        )
                    # fp8 PT (P*16 via exp bias shift+ln16, straight from
                    # PSUM) for the key-tiles phase C contracts in
                    # DoubleRow mode; the rowsum acc keeps using the fp16
                    # PT (quantization is unbiased; mismatch ~0.1%)
                    if st < N_C8_ST:
                        for sc in range(NQ):
                            nc.scalar.activation(
                                PT8s[sc][:, st, :],
                                pss[sc],
                                mybir.ActivationFunctionType.Exp,
                                bias=shift8_b[:, 0:1],
                                scale=1.0,
                            )
                    # running elementwise sum of PT tiles on the (idle) DVE
                    if st == 1:
                        nc.vector.tensor_tensor(
                            out=acc, in0=PT[0], in1=PT[1], op=mybir.AluOpType.add
                        )
                    elif st > 1:
                        nc.vector.tensor_tensor(
                            out=acc, in0=acc, in1=PT[st], op=mybir.AluOpType.add
                        )
                # ======== phase C: YT[d, s] = x-contracted P ===============
                # (row-sum pipeline emitted after C's first group so the two
                # ones-matmuls don't sit on the B->C critical path)
                # all fp8 DoubleRow groups in one contiguous block: the PE
                # pays the fp16<->DoubleRow array mode switch ~370ns each
                # time, so batching them costs 2 switches instead of 16.
                # Each dc's partial is staged to SBUF fp16 by the idle ACT
                # engine (adds ~1.5e-4 relative noise, negligible).
                if N_C8_ST:
                    for dc in range(DC):
                        ps8 = [
                            ps_c8.tile([P, 512], F32, tag="mm8", name="mm8")
                            for _ in range(NQ)
                        ]
                        for sp in range(N_C8_ST // 2):
                            for sc in range(NQ):
                                nc.tensor.matmul(
                                    ps8[sc],
                                    lhsT=xn8[:, dc, 2 * sp : 2 * sp + 2, :],
                                    rhs=PT8s[sc][:, 2 * sp : 2 * sp + 2, :],
                                    start=(sp == 0),
                                    stop=(sp == N_C8_ST // 2 - 1),
                                    perf_mode=mybir.MatmulPerfMode.DoubleRow,
                                )
                        for sc in range(NQ):
                            nc.scalar.activation(
                                yt8sb[NQ * dc + sc],
                                ps8[sc],
                                mybir.ActivationFunctionType.Identity,
                                scale=1.0,
                            )
                for dc in range(DC):
                    pss = [
                        ps_mm.tile([P, 512], F32, tag="mm", name="mm") for _ in range(NQ)
                    ]
                    for st in range(N_C8_ST, ST):
                        for sc in range(NQ):
                            nc.tensor.matmul(
                                pss[sc],
                                lhsT=xn[:, st, dc * P : (dc + 1) * P],
                                rhs=PT[st][:, sc * 512 : (sc + 1) * 512],
                                start=(st == N_C8_ST),
                                stop=(st == ST - 1),
                            )
                    for sc in range(NQ):
                        if N_C8_ST:
                            nc.vector.tensor_tensor(
                                out=YT[:, dc, sc * 512 : (sc + 1) * 512],
                                in0=pss[sc],
                                in1=yt8sb[NQ * dc + sc],
                                op=mybir.AluOpType.add,
                            )
                        else:
                            nc.vector.tensor_copy(
                                out=YT[:, dc, sc * 512 : (sc + 1) * 512],
                                in_=pss[sc],
                            )
                    if dc <= 1:
                        # cross-partition query totals: one tiny ones-matmul
                        # per 512-chunk (single psum-sum bank, reused one dc
                        # group apart so the reuse never stalls the PE), then
                        # [1, SQ] -> [128, SQT] via DRAM so sums line up with
                        # out partitions (SBUF APs cannot cross partitions)
                        sc = dc
                        pssum = ps_sum.tile([1, 512], F32, name="psum_s")
                        nc.tensor.matmul(
                            pssum,
                            lhsT=ones16,
                            rhs=acc[:, sc * 512 : (sc + 1) * 512],
                            start=True,
                            stop=True,
                        )
                        nc.vector.tensor_copy(
                            out=sums_sb[:, sc * 512 : (sc + 1) * 512],
                            in_=pssum,
                        )
                    if dc == 1:
                        sums_dram = dramtmp.tile([SQ], F32)
                        nc.sync.dma_start(out=sums_dram[None, :], in_=sums_sb)
                        nc.sync.dma_start(
                            out=sumsT, in_=sums_dram.rearrange("(t p) -> p t", p=P)
                        )
                        nc.vector.reciprocal(rsum, sumsT)

                # ======== phase D: out[s, o] = (YT^T U) * rsum + bprime ====
                o_sbs = [big.tile([P, D], F16, name=f"o_sb{i}") for i in range(2)]
                for sqt in range(SQT):
                    o_sb = o_sbs[sqt % 2]
                    pss = [
                        ps_mm.tile([P, 512], F32, tag="mm", name="mm")
                        for _ in range(2)
                    ]
                    for dc in range(DC):
                        for oc in range(2):
                            nc.tensor.matmul(
                                pss[oc],
                                lhsT=YT[:, dc, sqt * P : (sqt + 1) * P],
                                rhs=u16[:, oc, dc, :],
                                start=(dc == 0),
                                stop=(dc == DC - 1),
                            )
                    n_stt = 4 if sqt == SQT - 1 else 2
                    w_stt = D // n_stt
                    for ch in range(n_stt):
                        lo = ch * w_stt
                        nc.vector.scalar_tensor_tensor(
                            out=o_sb[:, lo : lo + w_stt],
                            in0=pss[lo // 512][:, lo % 512 : lo % 512 + w_stt],
                            scalar=rsum[:, sqt : sqt + 1],
                            in1=bp_b[:, lo : lo + w_stt],
                            op0=mybir.AluOpType.mult,
                            op1=mybir.AluOpType.add,
                        )
                    # alternate output queues so tiles drain on two engines
                    # in parallel; the last group goes in quarters so its
                    # exposed final transfer is half as long
                    for oc in range(2):
                        if sqt == SQT - 1:
                            for h in range(2):
                                q = nc.sync if (2 * oc + h) % 2 == 0 else nc.scalar
                                lo = oc * 512 + h * 256
                                q.dma_start(
                                    out=out_d[
                                        sqt * P : (sqt + 1) * P, lo : lo + 256
                                    ],
                                    in_=o_sb[:, lo : lo + 256],
                                )
                        else:
                            q = nc.sync if oc == 0 else nc.scalar
                            q.dma_start(
                                out=out_d[
                                    sqt * P : (sqt + 1) * P, oc * 512 : (oc + 1) * 512
                                ],
                                in_=o_sb[:, oc * 512 : (oc + 1) * 512],
                            )

    nc.finalize()
    return nc


def kernel(x, Wqkv, bqkv, Wout, bout):
    from concourse.bass_utils import run_bass_kernel_spmd
    import ml_dtypes

    x = np.ascontiguousarray(x, dtype=np.float32)
    Wqkv = np.asarray(Wqkv, dtype=np.float32)
    bqkv = np.asarray(bqkv, dtype=np.float32)
    Wout = np.asarray(Wout, dtype=np.float32)
    bout = np.asarray(bout, dtype=np.float32)
    B = x.shape[0]
    scale = float(D) ** -0.5

    Wq, Wk, Wv = Wqkv[:, 0:D], Wqkv[:, D : 2 * D], Wqkv[:, 2 * D : 3 * D]
    bq, bk, bv = bqkv[0:D], bqkv[D : 2 * D], bqkv[2 * D : 3 * D]

    # folded weights (the dropped per-query logit terms cancel in softmax)
    M16 = (scale * (Wq @ Wk.T)).astype(np.float16)
    U16 = (Wv @ Wout).astype(np.float16)
    zbias = np.ascontiguousarray(scale * (bq @ Wk.T), dtype=np.float32)
    bprime = np.ascontiguousarray(bv @ Wout + bout, dtype=np.float32)

    # pack into per-partition-contiguous SBUF layouts
    Mp = M16.reshape(DC, P, DC, P).transpose(1, 0, 2, 3)  # (p, dc, jb, jw)
    mj0 = np.ascontiguousarray(Mp[:, :, 0, :])
    mj1 = np.ascontiguousarray(Mp[:, :, 1, :])
    mj23 = np.ascontiguousarray(Mp[:, :, 2:4, :].reshape(P, DC, 2 * P))
    mj47 = np.ascontiguousarray(Mp[:, :, 4:8, :].reshape(P, DC, 4 * P))
    U16p = np.ascontiguousarray(
        U16.reshape(DC, P, 2, 512).transpose(2, 1, 0, 3)
    )  # (oc, p, dc, ow)

    nc = _build_core_program()

    in_maps = []
    for c in range(8):
        b, h = c // 2, c % 2
        # rotate the sequence so this core's queries are rows 0:SQ
        xs = np.concatenate([x[b, h * SQ :], x[b, : h * SQ]], axis=0)
        xn16 = np.ascontiguousarray(xs.astype(np.float16))
        xT = xn16.T  # [D, S]
        im = {
                "xta": np.ascontiguousarray(
                    xT[:, 0:512].reshape(DC, P, 512).transpose(1, 0, 2)
                ),
                "xtb": np.ascontiguousarray(
                    xT[:, 512:1024].reshape(DC, P, 512).transpose(1, 0, 2)
                ),
                "xtc": np.ascontiguousarray(
                    xT[:, 1024:2048].reshape(DC, P, 1024).transpose(1, 0, 2)
                ),
                "xn16": xn16,
                "mj0": mj0,
                "mj1": mj1,
                "mj23": mj23,
                "mj47": mj47,
                "U16p": U16p,
                "zbias": zbias,
                "bprime": bprime,
        }
        if N_C8_ST:
            x8 = np.clip(
                xs[: N_C8_ST * P].astype(np.float32) / P8_SCALE, -240.0, 240.0
            ).astype(ml_dtypes.float8_e4m3)
            im["xn8"] = np.ascontiguousarray(
                x8.reshape(N_C8_ST, P, DC, P).transpose(1, 2, 0, 3)
            )
        in_maps.append(im)

    res = run_bass_kernel_spmd(nc, in_maps, core_ids=list(range(8)))

    out = np.empty((B, S, D), dtype=np.float32)
    for c in range(8):
        b, h = c // 2, c % 2
        out[b, h * SQ : (h + 1) * SQ, :] = res.results[c]["out"].astype(np.float32)
    return out



# revision 27
# speedup vs baseline: 1.0198x; 1.0180x over previous
"""Trainium2 Bass kernel for single-head attention:
out = softmax((x@Wq)(x@Wk)^T / sqrt(D)) (x@Wv) @ Wout + bout, per batch.

Shapes: x [4, 2048, 1024], Wqkv [1024, 3072], Wout [1024, 1024].

Weight folding (host-side, exact):
  logits = (x Wq + bq)(x Wk + bk)^T * s
         = x (s Wq Wk^T) x^T + (s bq Wk^T) x^T + per-query-consts
  The per-query constant terms cancel in softmax, so with
  M = s*Wq@Wk^T and zbias = s*bq@Wk^T:  logits ~ (x@M + zbias) @ x^T.
  Similarly out = A (x Wv + bv) Wout + bout with A row-stochastic gives
  out = (P @ x @ U) / rowsum(P) + bprime, U = Wv@Wout, bprime = bv@Wout+bout.
This removes the K and V projections and the separate out-projection:
12.9 GFLOP/core (768 fp16-equivalent matmuls) instead of 21.5 GFLOP/core.

Sharding: 8 cores = 4 batches x 2 query-halves. Each core gets the full
sequence of its batch (rotated so its 1024 queries are rows 0:1023 --
attention is permutation-invariant over key/value positions) and writes
a [1024, 1024] slice of the output. No collectives.

All inputs are host-packed into the exact SBUF layouts so every load is
one dma_start with fully-contiguous per-partition reads. 17 HAM-warmup
matmuls bridge the ~13us until the first inputs land (the DMA engines
ramp slowly: ~50 GB/s for the first ~10us, ~350 GB/s after).

Device phases (matmuls fp16 with fp32 PSUM accumulation, except the
fp8 part of C):
  A: zT[d',s]  = M^T x_local^T        (128 mm)
  B: PT[t,s]   = exp(x z^T + shift)   (256 mm), running DVE row-sum acc;
     for the first N_C8_ST key-tiles a second exp (bias shift+ln16)
     writes PT8 = e4m3(P*16) straight from PSUM
  C: YT[d,s]   = x-contracted P: the first N_C8_ST key-tiles as fp8
     DoubleRow matmuls (contract 2 key-tiles per instruction at fp16
     cost -- 2x) against xn8 = e4m3(x/16) into a separate PSUM bank
     (mixing perf modes inside one accumulation group crashes the PE;
     a partial-inner-width DoubleRow rhs slice deadlocks scheduling, so
     PT8 is stored per-sc full-width), rest fp16; ACT stages the fp8
     psum to SBUF, DVE adds it into the YT write. PT8*xn8 = P*x
     natively, so fp8/fp16 partials add without rescaling. The DR groups
     use a dedicated 2-bank PSUM pool so their group-start allocation
     is always freed by the fast ACT copy, not a slow DVE add.
  D: out[s,o]  = (YT^T U) * rsum + bprime             (128 mm)
Softmax exp uses a constant -4 logit shift so fp16 never overflows
(the shift cancels in the normalization); rowsum uses the fp16 PT
(e4m3 rounding is ~unbiased, the mismatch is ~0.1% of the fp8 noise).

N_C8_ST=6 gives rel err 1.66e-2 (vs 6e-4 all-fp16, 2e-2 budget),
measured bit-matching the numpy e4m3 simulation, and ~10us of PE time.
Outputs alternate the sync/scalar DMA queues; the last tile's STT and
DMA go in quarters so the exposed final chain is short.

Observed hazard: the chip sporadically enters power state P0 (whole
core -1/6 clock, 216->259ns per matmul) for a full run -- environmental
and not caused by this program (same binary measured 177us and 211us).
"""

import sys

if "/opt/trn_rl_repo" not in sys.path:
    sys.path.insert(0, "/opt/trn_rl_repo")

import numpy as np

import concourse.mybir as mybir
from concourse import bacc
from concourse.tile import TileContext

P = 128
D = 1024          # d_model
S = 2048          # full sequence per batch
SQ = 1024         # queries per core
DC = D // P       # 8 d-chunks
ST = S // P       # 16 sequence tiles
NQ = SQ // 512    # 2 query 512-chunks
SQT = SQ // P     # 8 query 128-tiles

F32 = mybir.dt.float32
F16 = mybir.dt.float16
F8 = mybir.dt.float8e4
EXP_SHIFT = -4.0  # softmax logit shift (cancels in normalization)

# key st-tiles of phase C computed in fp8 DoubleRow (must be even).
# rel-err (device-exact sim): 0->6e-4, 2->9.7e-3, 4->1.35e-2, 6->1.65e-2
N_C8_ST = 8
P8_SCALE = 16.0   # PT8 = e4m3(P*16), xn8 = e4m3(x/16)


def _build_core_program():
    nc = bacc.Bacc()

    # host-packed inputs: leading axis 128 = SBUF partition, rest contiguous
    xta_d = nc.dram_tensor("xta", [P, DC, 512], F16, kind="ExternalInput")
    xtb_d = nc.dram_tensor("xtb", [P, DC, 512], F16, kind="ExternalInput")
    xtc_d = nc.dram_tensor("xtc", [P, DC, 1024], F16, kind="ExternalInput")
    xn_d = nc.dram_tensor("xn16", [S, D], F16, kind="ExternalInput")
    mj0_d = nc.dram_tensor("mj0", [P, DC, P], F16, kind="ExternalInput")
    mj1_d = nc.dram_tensor("mj1", [P, DC, P], F16, kind="ExternalInput")
    mj23_d = nc.dram_tensor("mj23", [P, DC, 2 * P], F16, kind="ExternalInput")
    mj47_d = nc.dram_tensor("mj47", [P, DC, 4 * P], F16, kind="ExternalInput")
    u_d = nc.dram_tensor("U16p", [2, P, DC, 512], F16, kind="ExternalInput")
    zb_d = nc.dram_tensor("zbias", [D], F32, kind="ExternalInput")
    bp_d = nc.dram_tensor("bprime", [D], F32, kind="ExternalInput")
    if N_C8_ST:
        xn8_d = nc.dram_tensor(
            "xn8", [P, DC, N_C8_ST, P], F8, kind="ExternalInput"
        )
    out_d = nc.dram_tensor("out", [SQ, D], F16, kind="ExternalOutput")

    with TileContext(nc) as tc:
        with (
            tc.tile_pool(name="const", bufs=1) as const,
            tc.tile_pool(name="ps_mm", bufs=5, space="PSUM") as ps_mm,
            tc.tile_pool(name="ps_c8", bufs=2, space="PSUM") as ps_c8,
            tc.tile_pool(name="ps_sum", bufs=1, space="PSUM") as ps_sum,
            tc.tile_pool(name="dramtmp", bufs=1, space="DRAM") as dramtmp,
        ):
            # warm memset on GpSimd (ready ~1us before DVE) so HAM warmup
            # matmuls start ASAP
            warm = const.tile([P, 512], F16)
            nc.gpsimd.memset(warm, 0.0)
            ones16 = const.tile([P, 1], F16)
            nc.vector.memset(ones16, 1.0)
            shift_b = const.tile([P, 1], F32)
            nc.vector.memset(shift_b, EXP_SHIFT)
            shift8_b = const.tile([P, 1], F32)
            nc.vector.memset(shift8_b, EXP_SHIFT + float(np.log(P8_SCALE)))
            sums_sb = const.tile([1, SQ], F32)
            sumsT = const.tile([P, SQT], F32)
            rsum = const.tile([P, SQT], F32)

            with tc.tile_pool(name="big", bufs=1) as big:
                xta = big.tile([P, DC, 512], F16)
                xtb = big.tile([P, DC, 512], F16)
                xtc = big.tile([P, DC, 1024], F16)
                xn = big.tile([P, ST, D], F16)
                m_t = big.tile([P, DC, D], F16)
                u16 = big.tile([P, 2, DC, 512], F16)
                zb = const.tile([P, DC], F32)
                bp_b = const.tile([P, D], F32)
                zT = big.tile([P, DC, SQ], F16)
                PT = [big.tile([P, SQ], F16, name=f"PT{st}") for st in range(ST)]
                acc = big.tile([P, SQ], F16, name="pt_acc")
                YT = big.tile([P, DC, SQ], F16)
                if N_C8_ST:
                    xn8 = big.tile([P, DC, N_C8_ST, P], F8)
                    yt8tmp = [
                        big.tile([P, 512], F32, name=f"yt8tmp{i}")
                        for i in range(2 * NQ)
                    ]
                    PT8s = [
                        big.tile([P, N_C8_ST, 512], F8, name=f"PT8_{sc}")
                        for sc in range(NQ)
                    ]

                # ---- input DMAs, ordered for earliest compute start --------
                # first M chunk + late-needed consts on the gpsimd queue,
                # everything else on sync in consumption order
                nc.sync.dma_start(out=m_t[:, :, 0:P], in_=mj0_d[:, :, :])
                nc.sync.dma_start(out=xta, in_=xta_d[:, :, :])
                nc.sync.dma_start(
                    out=zb, in_=zb_d.rearrange("(j p) -> p j", p=P)
                )
                nc.sync.dma_start(out=m_t[:, :, P : 2 * P], in_=mj1_d[:, :, :])
                nc.sync.dma_start(out=m_t[:, :, 2 * P : 4 * P], in_=mj23_d[:, :, :])
                nc.sync.dma_start(out=m_t[:, :, 4 * P : 8 * P], in_=mj47_d[:, :, :])
                nc.sync.dma_start(out=xtb, in_=xtb_d[:, :, :])
                nc.sync.dma_start(out=xtc, in_=xtc_d[:, :, :])
                for g in range(4):
                    nc.sync.dma_start(
                        out=xn[:, g * 4 : (g + 1) * 4, :],
                        in_=xn_d[g * 512 : (g + 1) * 512, :].rearrange(
                            "(st p) d -> p st d", p=P
                        ),
                    )
                if N_C8_ST:
                    nc.sync.dma_start(out=xn8, in_=xn8_d[:, :, :, :])
                for oc in range(2):
                    nc.sync.dma_start(out=u16[:, oc, :, :], in_=u_d[oc, :, :, :])
                # 512KB broadcast write hogs one DMA engine for ~16us --
                # keep it out of the critical early window (needed in D only)
                nc.sync.dma_start(
                    out=bp_b, in_=bp_d[None, :].to_broadcast([P, D])
                )

                # HAM warmup: keep the PE busy while the first inputs stream
                # in, so real matmuls start at 2.4 GHz instead of 1.2 GHz
                ps_warms = [
                    ps_mm.tile([P, 512], F32, tag="mm", name="ps_warm")
                    for _ in range(2)
                ]
                for i in range(17):
                    nc.tensor.matmul(
                        ps_warms[i % 2],
                        lhsT=warm[:, 0:P],
                        rhs=warm,
                        start=True,
                        stop=True,
                    )

                def xt_lhs(dc, st):
                    """lhsT slice [128 d, 128 t] of x^T for key tile st."""
                    if st < 4:
                        return xta[:, dc, st * P : (st + 1) * P]
                    if st < 8:
                        return xtb[:, dc, (st - 4) * P : (st - 3) * P]
                    return xtc[:, dc, (st - 8) * P : (st - 7) * P]

                # ======== phase A: zT[d', s] = M^T xq^T + zbias ============
                # sc outer: the first 64 matmuls touch only xta
                for sc in range(NQ):
                    xsrc = xta if sc == 0 else xtb
                    for j in range(DC):
                        ps = ps_mm.tile([P, 512], F32, tag="mm", name="mm")
                        for dc in range(DC):
                            nc.tensor.matmul(
                                ps,
                                lhsT=m_t[:, dc, j * P : (j + 1) * P],
                                rhs=xsrc[:, dc, :],
                                start=(dc == 0),
                                stop=(dc == DC - 1),
                            )
                        nc.scalar.activation(
                            zT[:, j, sc * 512 : (sc + 1) * 512],
                            ps,
                            mybir.ActivationFunctionType.Identity,
                            bias=zb[:, j : j + 1],
                            scale=1.0,
                        )

                # ======== phase B: PT[t, s] = exp(x z^T + shift) ===========
                for st in range(ST):
                    pss = [
                        ps_mm.tile([P, 512], F32, tag="mm", name="mm") for _ in range(NQ)
                    ]
                    for dc in range(DC):
                        for sc in range(NQ):
                            nc.tensor.matmul(
                                pss[sc],
                                lhsT=xt_lhs(dc, st),
                                rhs=zT[:, dc, sc * 512 : (sc + 1) * 512],
                                start=(dc == 0),
                                stop=(dc == DC - 1),
                            )
                    for sc in range(NQ):
                        nc.scalar.activation(
                            PT[st][:, sc * 512 : (sc + 1) * 512],
                            pss[sc],
                            mybir.ActivationFunctionType.Exp,
                            bias=shift_b[:, 0:1],
                            scale=1.0,
                        )
                    # fp8 PT (P*16 via exp bias shift+ln16, straight from
                    # PSUM) for the key-tiles phase C contracts in
                    # DoubleRow mode; the rowsum acc keeps using the fp16
                    # PT (quantization is unbiased; mismatch ~0.1%)
                    if st < N_C8_ST:
                        for sc in range(NQ):
                            nc.scalar.activation(
                                PT8s[sc][:, st, :],
                                pss[sc],
                                mybir.ActivationFunctionType.Exp,
                                bias=shift8_b[:, 0:1],
                                scale=1.0,
                            )
                    # running elementwise sum of PT tiles on the (idle) DVE
                    if st == 1:
                        nc.vector.tensor_tensor(
                            out=acc, in0=PT[0], in1=PT[1], op=mybir.AluOpType.add
                        )
                    elif st > 1:
                        nc.vector.tensor_tensor(
                            out=acc, in0=acc, in1=PT[st], op=mybir.AluOpType.add
                        )
                # ======== phase C: YT[d, s] = x-contracted P ===============
                # (row-sum pipeline emitted after C's first group so the two
                # ones-matmuls don't sit on the B->C critical path)
                for dc in range(DC):
                    pss = [
                        ps_mm.tile([P, 512], F32, tag="mm", name="mm") for _ in range(NQ)
                    ]
                    if N_C8_ST:
                        # dedicated banks: freed by the (fast) ACT copies so
                        # the next group's DR start never waits on the slower
                        # DVE-add-freed fp16 banks
                        ps8 = [
                            ps_c8.tile([P, 512], F32, tag="mm8", name="mm8")
                            for _ in range(NQ)
                        ]
                        for sp in range(N_C8_ST // 2):
                            for sc in range(NQ):
                                nc.tensor.matmul(
                                    ps8[sc],
                                    lhsT=xn8[:, dc, 2 * sp : 2 * sp + 2, :],
                                    rhs=PT8s[sc][:, 2 * sp : 2 * sp + 2, :],
                                    start=(sp == 0),
                                    stop=(sp == N_C8_ST // 2 - 1),
                                    perf_mode=mybir.MatmulPerfMode.DoubleRow,
                                )
                        for sc in range(NQ):
                            # idle ACT engine does the psum->sbuf stage so
                            # DVE only has the adds after each group stop
                            nc.scalar.activation(
                                yt8tmp[NQ * (dc % 2) + sc],
                                ps8[sc],
                                mybir.ActivationFunctionType.Identity,
                                scale=1.0,
                            )
                    for st in range(N_C8_ST, ST):
                        for sc in range(NQ):
                            nc.tensor.matmul(
                                pss[sc],
                                lhsT=xn[:, st, dc * P : (dc + 1) * P],
                                rhs=PT[st][:, sc * 512 : (sc + 1) * 512],
                                start=(st == N_C8_ST),
                                stop=(st == ST - 1),
                            )
                    for sc in range(NQ):
                        if N_C8_ST:
                            nc.vector.tensor_tensor(
                                out=YT[:, dc, sc * 512 : (sc + 1) * 512],
                                in0=pss[sc],
                                in1=yt8tmp[NQ * (dc % 2) + sc],
                                op=mybir.AluOpType.add,
                            )
                        else:
                            nc.vector.tensor_copy(
                                out=YT[:, dc, sc * 512 : (sc + 1) * 512],
                                in_=pss[sc],
                            )
                    if dc <= 1:
                        # cross-partition query totals: one tiny ones-matmul
                        # per 512-chunk (single psum-sum bank, reused one dc
                        # group apart so the reuse never stalls the PE), then
                        # [1, SQ] -> [128, SQT] via DRAM so sums line up with
                        # out partitions (SBUF APs cannot cross partitions)
                        sc = dc
                        pssum = ps_sum.tile([1, 512], F32, name="psum_s")
                        nc.tensor.matmul(
                            pssum,
                            lhsT=ones16,
                            rhs=acc[:, sc * 512 : (sc + 1) * 512],
                            start=True,
                            stop=True,
                        )
                        nc.vector.tensor_copy(
                            out=sums_sb[:, sc * 512 : (sc + 1) * 512],
                            in_=pssum,
                        )
                    if dc == 1:
                        sums_dram = dramtmp.tile([SQ], F32)
                        nc.sync.dma_start(out=sums_dram[None, :], in_=sums_sb)
                        nc.sync.dma_start(
                            out=sumsT, in_=sums_dram.rearrange("(t p) -> p t", p=P)
                        )
                        nc.vector.reciprocal(rsum, sumsT)

                # ======== phase D: out[s, o] = (YT^T U) * rsum + bprime ====
                o_sbs = [big.tile([P, D], F16, name=f"o_sb{i}") for i in range(2)]
                for sqt in range(SQT):
                    o_sb = o_sbs[sqt % 2]
                    pss = [
                        ps_mm.tile([P, 512], F32, tag="mm", name="mm")
                        for _ in range(2)
                    ]
                    for dc in range(DC):
                        for oc in range(2):
                            nc.tensor.matmul(
                                pss[oc],
                                lhsT=YT[:, dc, sqt * P : (sqt + 1) * P],
                                rhs=u16[:, oc, dc, :],
                                start=(dc == 0),
                                stop=(dc == DC - 1),
                            )
                    n_stt = 4 if sqt == SQT - 1 else 2
                    w_stt = D // n_stt
                    for ch in range(n_stt):
                        lo = ch * w_stt
                        nc.vector.scalar_tensor_tensor(
                            out=o_sb[:, lo : lo + w_stt],
                            in0=pss[lo // 512][:, lo % 512 : lo % 512 + w_stt],
                            scalar=rsum[:, sqt : sqt + 1],
                            in1=bp_b[:, lo : lo + w_stt],
                            op0=mybir.AluOpType.mult,
                            op1=mybir.AluOpType.add,
                        )
                    # alternate output queues so tiles drain on two engines
                    # in parallel; the last group goes in quarters so its
                    # exposed final transfer is half as long
                    for oc in range(2):
                        if sqt == SQT - 1:
                            for h in range(2):
                                q = nc.sync if (2 * oc + h) % 2 == 0 else nc.scalar
                                lo = oc * 512 + h * 256
                                q.dma_start(
                                    out=out_d[
                                        sqt * P : (sqt + 1) * P, lo : lo + 256
                                    ],
                                    in_=o_sb[:, lo : lo + 256],
                                )
                        else:
                            q = nc.sync if oc == 0 else nc.scalar
                            q.dma_start(
                                out=out_d[
                                    sqt * P : (sqt + 1) * P, oc * 512 : (oc + 1) * 512
                                ],
                                in_=o_sb[:, oc * 512 : (oc + 1) * 512],
                            )

    nc.finalize()
    return nc


def kernel(x, Wqkv, bqkv, Wout, bout):
    from concourse.bass_utils import run_bass_kernel_spmd
    import ml_dtypes

    x = np.ascontiguousarray(x, dtype=np.float32)
    Wqkv = np.asarray(Wqkv, dtype=np.float32)
    bqkv = np.asarray(bqkv, dtype=np.float32)
    Wout = np.asarray(Wout, dtype=np.float32)
    bout = np.asarray(bout, dtype=np.float32)
    B = x.shape[0]
    scale = float(D) ** -0.5

    Wq, Wk, Wv = Wqkv[:, 0:D], Wqkv[:, D : 2 * D], Wqkv[:, 2 * D : 3 * D]
    bq, bk, bv = bqkv[0:D], bqkv[D : 2 * D], bqkv[2 * D : 3 * D]

    # folded weights (the dropped per-query logit terms cancel in softmax)
    M16 = (scale * (Wq @ Wk.T)).astype(np.float16)
    U16 = (Wv @ Wout).astype(np.float16)
    zbias = np.ascontiguousarray(scale * (bq @ Wk.T), dtype=np.float32)
    bprime = np.ascontiguousarray(bv @ Wout + bout, dtype=np.float32)

    # pack into per-partition-contiguous SBUF layouts
    Mp = M16.reshape(DC, P, DC, P).transpose(1, 0, 2, 3)  # (p, dc, jb, jw)
    mj0 = np.ascontiguousarray(Mp[:, :, 0, :])
    mj1 = np.ascontiguousarray(Mp[:, :, 1, :])
    mj23 = np.ascontiguousarray(Mp[:, :, 2:4, :].reshape(P, DC, 2 * P))
    mj47 = np.ascontiguousarray(Mp[:, :, 4:8, :].reshape(P, DC, 4 * P))
    U16p = np.ascontiguousarray(
        U16.reshape(DC, P, 2, 512).transpose(2, 1, 0, 3)
    )  # (oc, p, dc, ow)

    nc = _build_core_program()

    in_maps = []
    for c in range(8):
        b, h = c // 2, c % 2
        # rotate the sequence so this core's queries are rows 0:SQ
        xs = np.concatenate([x[b, h * SQ :], x[b, : h * SQ]], axis=0)
        xn16 = np.ascontiguousarray(xs.astype(np.float16))
        xT = xn16.T  # [D, S]
        im = {
                "xta": np.ascontiguousarray(
                    xT[:, 0:512].reshape(DC, P, 512).transpose(1, 0, 2)
                ),
                "xtb": np.ascontiguousarray(
                    xT[:, 512:1024].reshape(DC, P, 512).transpose(1, 0, 2)
                ),
                "xtc": np.ascontiguousarray(
                    xT[:, 1024:2048].reshape(DC, P, 1024).transpose(1, 0, 2)
                ),
                "xn16": xn16,
                "mj0": mj0,
                "mj1": mj1,
                "mj23": mj23,
                "mj47": mj47,
                "U16p": U16p,
                "zbias": zbias,
                "bprime": bprime,
        }
        if N_C8_ST:
            x8 = np.clip(
                xs[: N_C8_ST * P].astype(np.float32) / P8_SCALE, -240.0, 240.0
            ).astype(ml_dtypes.float8_e4m3)
            im["xn8"] = np.ascontiguousarray(
                x8.reshape(N_C8_ST, P, DC, P).transpose(1, 2, 0, 3)
            )
        in_maps.append(im)

    res = run_bass_kernel_spmd(nc, in_maps, core_ids=list(range(8)))

    out = np.empty((B, S, D), dtype=np.float32)
    for c in range(8):
        b, h = c // 2, c % 2
        out[b, h * SQ : (h + 1) * SQ, :] = res.results[c]["out"].astype(np.float32)
    return out

